# revision 23
# baseline (speedup 1.0000x reference)
"""MultiHeadLatentAttention Trainium2 kernel (8 NeuronCores, SPMD).

Sharding: core c -> (batch b = c // 4, latent group g = c % 4).
Each core owns query heads [4g, 4g+4) and latent head g for its batch:
  - q projection for its 4 heads (1/sqrt(HD) folded into the weights)
  - k, v via HOST-MERGED latent weights: k = x @ (kr_w@kl_w)^T + (kr_w@kl_b
    + kr_b) -- mathematically identical to the two-stage latent form but
    4.25x fewer FLOPs since head_dim(128) << latent_dim(512)
  - causal attention in transposed layout (scoresT[s_k, s_q]) processed in
    128-row q blocks with ALL FOUR HEADS fused into one [128, 4*128] moving
    operand per matmul: exact causality at 128-block granularity (no
    diagonal wedge waste) and the diagonal mask is one [128,512] multiply
    with a 4x-tiled 0/1 tril mask.
  - softmax denominators: prob blocks are pairwise-summed on the DVE (bf16
    binomial tree, one rounding of the total) and a single ones [128x128]
    stationary matmul per q-block does the partition reduction + broadcast;
    this removes the per-block PE sum matmuls entirely.
  - projection matmuls for chunk n+1 and o_proj matmuls for chunk n-1 are
    emitted between attention units so the PE never stalls on the ScalarE
    exp chain; each unit's AV tail + normalize is deferred one unit.
  - partial o_proj (its 512 input dims -> full 2048 output dims), bf16 out
Host sums the 4 partial o_proj outputs per batch (fp32) and adds o_b.

All matmuls run in bf16 with fp32 PSUM accumulation.
"""

import math

import numpy as np
import ml_dtypes

B, S, H = 2, 2048, 2048
NH, HD = 16, 128
NLH, LD = 4, 512
HPC = 4            # query heads per core
NCORES = 8
SQC = 512          # s_q chunk (4 fused-head q blocks of 128)
NQC = S // SQC     # 4 chunks
NKT = H // 128     # 16 contraction tiles for the projections
NSB = S // 128     # 16 s_k / s_q 128-blocks
BF16 = ml_dtypes.bfloat16

_CACHE = {}


def _build_program(repeat=1):
    import concourse.bacc as bacc
    import concourse.bass as bass
    import concourse.tile as tile
    from concourse import mybir
    from contextlib import ExitStack

    dt = mybir.dt
    AF = mybir.ActivationFunctionType

    nc = bacc.Bacc("TRN2", target_bir_lowering=False, debug=False,
                   num_devices=NCORES)

    xT = nc.declare_dram_parameter("xT", [H, S], dt.bfloat16, isOutput=False)
    qw = nc.declare_dram_parameter("qwT", [H, HPC * HD], dt.bfloat16, isOutput=False)
    kw = nc.declare_dram_parameter("kwT", [H, HD], dt.bfloat16, isOutput=False)
    vw = nc.declare_dram_parameter("vwT", [H, HD], dt.bfloat16, isOutput=False)
    ow = nc.declare_dram_parameter("owT", [HPC * HD, H], dt.bfloat16, isOutput=False)
    qb = nc.declare_dram_parameter("qb", [HPC * HD], dt.float32, isOutput=False)
    kb = nc.declare_dram_parameter("kb", [HD], dt.float32, isOutput=False)
    vb = nc.declare_dram_parameter("vb", [HD], dt.float32, isOutput=False)
    tri = nc.declare_dram_parameter("tri", [128, HPC * 128], dt.bfloat16, isOutput=False)
    outp = nc.declare_dram_parameter("out", [H, S], dt.bfloat16, isOutput=True)

    with tile.TileContext(nc) as tc, ExitStack() as ctx:
        const = ctx.enter_context(tc.tile_pool(name="const", bufs=1))
        probs_pool = ctx.enter_context(tc.tile_pool(name="probs", bufs=12))
        acc_pool = ctx.enter_context(tc.tile_pool(name="accp", bufs=10))
        attn_pool = ctx.enter_context(tc.tile_pool(name="attn", bufs=4))
        small = ctx.enter_context(tc.tile_pool(name="small", bufs=8))
        psum = ctx.enter_context(tc.tile_pool(name="psum", bufs=8, space="PSUM"))

        # ---------------- constants / weights ----------------
        # first x chunk (gpsimd queues) + q weights (sync queues) land first
        # so the PE can start ASAP; k-interleaved so (qw[k], xt[k]) pairs
        # arrive in consumption order.
        qw_sb = const.tile([128, NKT, HPC * HD], dt.bfloat16, tag="qw")
        kw_sb = const.tile([128, NKT, HD], dt.bfloat16, tag="kw")
        vw_sb = const.tile([128, NKT, HD], dt.bfloat16, tag="vw")
        for k4 in range(4):
            nc.sync.dma_start(
                out=kw_sb[:, 4 * k4:4 * (k4 + 1), :],
                in_=kw.ap()[512 * k4:512 * (k4 + 1), :]
                .rearrange("(k p) m -> p k m", p=128))
        for k4 in range(4):
            nc.sync.dma_start(
                out=vw_sb[:, 4 * k4:4 * (k4 + 1), :],
                in_=vw.ap()[512 * k4:512 * (k4 + 1), :]
                .rearrange("(k p) m -> p k m", p=128))
        # x stays RESIDENT in SBUF across repeat bodies (64KB/partition):
        # no per-body x DMA, no cross-body WAR chains. Chunk-0 columns land
        # first so the first body's projections can start early.
        xT_sb = const.tile([128, NKT, S], dt.bfloat16, tag="xT")
        for k in range(NKT):
            nc.gpsimd.dma_start(out=xT_sb[:, k, 0:SQC],
                                in_=xT.ap()[128 * k:128 * (k + 1), 0:SQC])
            eng = nc.sync if k < 6 else nc.gpsimd
            eng.dma_start(out=qw_sb[:, k, :], in_=qw.ap()[128 * k:128 * (k + 1), :])
        for n in range(1, NQC):
            for k in range(NKT):
                eng = nc.sync if k % 2 == 0 else nc.gpsimd
                eng.dma_start(
                    out=xT_sb[:, k, SQC * n:SQC * (n + 1)],
                    in_=xT.ap()[128 * k:128 * (k + 1), SQC * n:SQC * (n + 1)])

        qb_sb = const.tile([128, HPC], dt.float32, tag="qb")
        nc.sync.dma_start(out=qb_sb, in_=qb.ap().rearrange("(m p) -> p m", p=128))
        kb_sb = const.tile([128, 1], dt.float32, tag="kb")
        nc.sync.dma_start(out=kb_sb, in_=kb.ap().rearrange("(m p) -> p m", p=128))

        vb_ap = vb.ap()
        vb_bc = const.tile([128, HD], dt.float32, tag="vbb")
        nc.sync.dma_start(
            out=vb_bc,
            in_=bass.AP(tensor=vb_ap.tensor, offset=vb_ap.offset,
                        ap=[[0, 128]] + list(vb_ap.ap)),
        )
        tri4_sb = const.tile([128, HPC * 128], dt.bfloat16, tag="tri")
        nc.sync.dma_start(out=tri4_sb, in_=tri.ap())
        ones_sb = const.tile([128, 128], dt.bfloat16, tag="ones")
        nc.vector.memset(ones_sb, 1.0)
        # o_proj weights: first consumed late (o_proj of chunk 0, ~50us in),
        # so they ride the gpsimd queue behind the startup-critical DMAs.
        ow_sb = const.tile([128, HPC, H], dt.bfloat16, tag="ow")
        nc.gpsimd.dma_start(
            out=ow_sb, in_=ow.ap().rearrange("(k p) m -> p k m", p=128))

        # persistent activations
        # qT4: head-major [hd, head, s] so per-head proj writes are
        # contiguous and the fused score moving operand is a strided view.
        qT4_sb = const.tile([128, HPC, S], dt.bfloat16, tag="qT4")
        kT_sb = const.tile([128, S], dt.bfloat16, tag="kT")
        v_sb = const.tile([128, NSB, HD], dt.bfloat16, tag="v")

        from collections import deque

        # PE filler work-queues: items are closures emitting ~1-4 matmuls
        # each; they are pulled between score matmuls (and between units)
        # so the PE always has independent work while the exp chain and
        # the DVE prob-sum tree drain. projq (next chunk's projections)
        # drains with priority and must be empty before the chunk that
        # consumes those activations starts; oq (o_proj of a finished
        # chunk) may spill one chunk further.
        projq = deque()
        oq = deque()

        def pull(k):
            n = 0
            while n < k and (projq or oq):
                (projq or oq).popleft()()
                n += 1

        def qlen():
            return len(projq) + len(oq)

        def _emit_body(last):
            def proj_items(n):
                items = []

                def k_part(ps, k0, k1):
                    for k in range(k0, k1):
                        nc.tensor.matmul(ps, lhsT=kw_sb[:, k, :], rhs=xT_sb[:, k, SQC * n:SQC * (n + 1)],
                                         start=(k == 0), stop=(k == NKT - 1))
                    if k1 == NKT:
                        nc.scalar.activation(out=kT_sb[:, SQC * n:SQC * (n + 1)],
                                             in_=ps, func=AF.Identity,
                                             bias=kb_sb[:, 0:1])

                def v_part(ps, jj, k0, k1):
                    for k in range(k0, k1):
                        nc.tensor.matmul(ps[:, :HD],
                                         lhsT=xT_sb[:, k, SQC * n + 128 * jj:SQC * n + 128 * (jj + 1)],
                                         rhs=vw_sb[:, k, :],
                                         start=(k == 0), stop=(k == NKT - 1))
                    if k1 == NKT:
                        nc.vector.tensor_add(out=v_sb[:, 4 * n + jj, :],
                                             in0=ps[:, :HD], in1=vb_bc)

                def q_part(ps, h, k0, k1):
                    for k in range(k0, k1):
                        nc.tensor.matmul(ps,
                                         lhsT=qw_sb[:, k, 128 * h:128 * (h + 1)],
                                         rhs=xT_sb[:, k, SQC * n:SQC * (n + 1)],
                                         start=(k == 0), stop=(k == NKT - 1))
                    if k1 == NKT:
                        nc.scalar.activation(
                            out=qT4_sb[:, h, SQC * n:SQC * (n + 1)], in_=ps,
                            func=AF.Identity, bias=qb_sb[:, h:h + 1])

                def group(part, nparts, *args):
                    st = {}

                    def mk(i):
                        def item():
                            if "ps" not in st:
                                st["ps"] = psum.tile([128, SQC], dt.float32,
                                                     tag="bank", name="ps_p")
                            kk = NKT // nparts
                            part(st["ps"], *args, i * kk, (i + 1) * kk)
                        return item
                    return [mk(i) for i in range(nparts)]

                items += group(k_part, 4)
                for jj in range(4):
                    items += group(v_part, 2, jj)
                for h in range(HPC):
                    items += group(q_part, 4, h)
                return items

            def o_items(n, at_tiles, m0, m1):
                def mk(m):
                    def item():
                        ps_o = psum.tile([128, SQC], dt.float32, tag="bank",
                                         name="ps_o")
                        for h in range(HPC):
                            nc.tensor.matmul(ps_o,
                                             lhsT=ow_sb[:, h, 128 * m:128 * (m + 1)],
                                             rhs=at_tiles[:, h, :],
                                             start=(h == 0), stop=(h == 3))
                        o_sb = small.tile([128, SQC], dt.bfloat16, tag="osb",
                                          name="osb")
                        if m % 2 == 0:
                            nc.scalar.copy(out=o_sb, in_=ps_o)
                        else:
                            if m % 2 == 0:
                        nc.scalar.copy(out=o_sb, in_=ps_o)
                    else:
                        nc.vector.tensor_copy(out=o_sb, in_=ps_o)
                        nc.sync.dma_start(
                            out=outp.ap()[128 * m:128 * (m + 1),
                                          SQC * n:SQC * (n + 1)],
                            in_=o_sb)
                    return item
                return [mk(m) for m in range(m0, m1)]

            # attention unit for fused-head q block g: scoresT/probs are
            # [s_k=128, (head, q128)=512].
            def make_unit(g, at_tiles):
                J = g + 1
                state = {"av": None}
                pending = []   # (j, pt) awaiting the AV matmul
                stack = []     # (level, tile) binomial tree of prob sums

                def emit_av(j, pt):
                    if state["av"] is None:
                        state["av"] = psum.tile([128, SQC], dt.float32,
                                                tag="bank", name="ps_av")
                    nc.tensor.matmul(state["av"], lhsT=v_sb[:, j, :], rhs=pt,
                                     start=(j == 0), stop=(j == J - 1))

                def tree_push(pt):
                    stack.append((0, pt))
                    while len(stack) >= 2 and stack[-1][0] == stack[-2][0]:
                        l2, b = stack.pop()
                        l1, a = stack.pop()
                        sm = acc_pool.tile([128, SQC], dt.bfloat16, tag="acc",
                                           name="acc")
                        nc.vector.tensor_add(out=sm, in0=a, in1=b)
                        stack.append((l1 + 1, sm))

                def emit_scores():
                    for j in range(J):
                        ps_s = psum.tile([128, SQC], dt.float32, tag="bank",
                                         name="ps_s")
                        nc.tensor.matmul(ps_s,
                                         lhsT=kT_sb[:, 128 * j:128 * (j + 1)],
                                         rhs=qT4_sb[:, :, 128 * g:128 * (g + 1)],
                                         start=True, stop=True)
                        pt = probs_pool.tile([128, SQC], dt.bfloat16, tag="pt",
                                             name="pt")
                        nc.scalar.activation(out=pt, in_=ps_s, func=AF.Exp)
                        if j == g:
                            nc.vector.tensor_mul(out=pt, in0=pt, in1=tri4_sb)
                        tree_push(pt)
                        pending.append((j, pt))
                        if len(pending) > 3:
                            emit_av(*pending.pop(0))
                        # beyond the PSUM ring depth the score stream outruns
                        # the exp chain (~357 ns/block); cover with filler
                        if j >= 5 and j % 2 == 1:
                            pull(1)

                def emit_tail():
                    for p in pending:
                        emit_av(*p)
                    while len(stack) > 1:
                        l2, b = stack.pop()
                        l1, a = stack.pop()
                        sm = acc_pool.tile([128, SQC], dt.bfloat16, tag="acc",
                                           name="acc")
                        nc.vector.tensor_add(out=sm, in0=a, in1=b)
                        stack.append((max(l1, l2) + 1, sm))
                    acc = stack[0][1]
                    d4 = psum.tile([128, SQC], dt.float32, tag="bank", name="ps_d")
                    nc.tensor.matmul(d4, lhsT=ones_sb, rhs=acc,
                                     start=True, stop=True)
                    recip = small.tile([128, SQC], dt.float32, tag="recip",
                                       name="recip")
                    nc.vector.reciprocal_approx_fast(out=recip, in_=d4)
                    qi = g % 4
                    for h in range(HPC):
                        nc.vector.tensor_mul(
                            out=at_tiles[h][:, 128 * qi:128 * (qi + 1)],
                            in0=state["av"][:, 128 * h:128 * (h + 1)],
                            in1=recip[:, 128 * h:128 * (h + 1)])

                return emit_scores, emit_tail

            # ---------------- body ----------------
            # chunk-0 projections were emitted by the previous body's tail
            # (or by the preamble for the first body).
            at_by_chunk = {}
            prev_tail = None
            for n in range(NQC):
                at_tiles = attn_pool.tile([128, HPC, SQC], dt.bfloat16,
                                          tag="at", name="at_all")
                at_by_chunk[n] = at_tiles

                if n < NQC - 1:
                    projq.extend(proj_items(n + 1))

                for s in range(4):
                    g = 4 * n + s
                    emit_scores, emit_tail = make_unit(g, at_tiles)
                    emit_scores()
                    if s == 3:
                        # chunk n+1's projections must be fully emitted
                        # before its first unit; keep o work in reserve.
                        pull(max(len(projq), qlen() // 2))
                    else:
                        pull(qlen() // (7 - s))
                    if prev_tail is not None:
                        prev_tail()
                    prev_tail = emit_tail
                    if s == 0 and n >= 1:
                        # at(n-1) is complete once tail(4n-1) is emitted
                        oq.extend(o_items(n - 1, at_by_chunk[n - 1], 0, 16))
            prev_tail()
            if not last:
                # next body's chunk-0 projections cover the tail(15) ->
                # at(3) serial chain and the final o_proj stretch
                projq.extend(proj_items(0))
            pull(3)
            for it in o_items(NQC - 1, at_by_chunk[NQC - 1], 0, 16):
                it()
                pull(1)
            pull(qlen())

        for it in proj_items(0):
            it()
        for _rep in range(repeat):
            _emit_body(last=(_rep == repeat - 1))

    nc.compile()
    return nc


def _get_nc(repeat=1):
    key = f"nc{repeat}"
    if key not in _CACHE:
        _CACHE[key] = _build_program(repeat)
    return _CACHE[key]


def _make_in_maps(hidden_states, attention_mask, q_w, q_b, kl_w, kl_b, vl_w, vl_b,
                  kr_w, kr_b, vr_w, vr_b, o_w):
    scale = 1.0 / math.sqrt(HD)
    tri01 = (np.asarray(attention_mask[0, 0, :128, :128]) == 0).T.astype(np.float32)
    tri4 = np.tile(tri01, (1, HPC)).astype(BF16)
    kr_f = np.asarray(kr_w, np.float32)
    vr_f = np.asarray(vr_w, np.float32)
    in_maps = []
    for c in range(NCORES):
        b, g = divmod(c, NLH)
        sl = slice(LD * g, LD * (g + 1))
        xTc = np.ascontiguousarray(np.asarray(hidden_states[b], np.float32).T
                                   ).astype(BF16)
        # merged latent->head weights: k = x @ (kr_w @ kl_w)^T + (kr_w@kl_b + kr_b)
        kw_eff = kr_f @ np.asarray(kl_w[sl], np.float32)
        vw_eff = vr_f @ np.asarray(vl_w[sl], np.float32)
        kb_eff = kr_f @ np.asarray(kl_b[sl], np.float32) + np.asarray(kr_b, np.float32)
        vb_eff = vr_f @ np.asarray(vl_b[sl], np.float32) + np.asarray(vr_b, np.float32)
        in_maps.append({
            "xT": xTc,
            "qwT": np.ascontiguousarray(
                (np.asarray(q_w[sl], np.float32) * scale).T).astype(BF16),
            "kwT": np.ascontiguousarray(kw_eff.T).astype(BF16),
            "vwT": np.ascontiguousarray(vw_eff.T).astype(BF16),
            "owT": np.ascontiguousarray(np.asarray(o_w, np.float32)[:, sl].T
                                        ).astype(BF16),
            "qb": (np.asarray(q_b[sl], np.float32) * scale),
            "kb": kb_eff,
            "vb": vb_eff,
            "tri": tri4,
        })
    return in_maps


def _gather(results, o_b):
    o_b = np.asarray(o_b, np.float32)
    outs = []
    for b in range(B):
        acc = np.zeros((H, S), np.float32)
        for g in range(NLH):
            acc += results[b * NLH + g]["out"].astype(np.float32)
        outs.append(acc.T + o_b[None, :])
    return np.stack(outs).astype(np.float32)


def kernel(hidden_states, position_ids, attention_mask, q_w, q_b, kl_w, kl_b,
           vl_w, vl_b, kr_w, kr_b, vr_w, vr_b, o_w, o_b):
    from concourse.bass_utils import run_bass_kernel_spmd

    nc = _get_nc()
    in_maps = _make_in_maps(hidden_states, attention_mask, q_w, q_b, kl_w, kl_b,
                            vl_w, vl_b, kr_w, kr_b, vr_w, vr_b, o_w)
    res = run_bass_kernel_spmd(nc, in_maps, core_ids=list(range(NCORES)))
    return _gather(res.results, o_b)
